# revision 1
# baseline (speedup 1.0000x reference)
"""Dynamic filter layer on 8 trn2 NeuronCores — v8 (DMA-layout optimized).

out[b,i,j,c] = sum_{di,dj} x[b,i+di,j+dj,c] * flow[b,i,j,di*K+dj]

B=8, H=W=256, C=64, K=5, Ho=Wo=252. Sharding: data-parallel over batch,
one sample per core (SPMD, no collectives).

v8 design, driven by HW ablation (v7 was DMA-skeleton-bound at ~630us,
~5us per fragmented 20-byte-run flow DMA; tensor ops only ~1.1us
marginal each because the dup-pair packed DVE path is fast):
  - Host repacks flow per (row-block, col-chunk) into fdc
    [NCHUNK, 128, K, JW, 2K] bf16: the K row-shifted dup'd tap-group
    tiles with edge rows pre-zeroed, so each chunk needs ONE fully
    contiguous flow DMA (6.4KB/partition) instead of five 20B-run DMAs
    (and no memsets / edge branches).
  - Flow factor read via dup-pair AP [jw, C/2 x stride-0, 2 x stride-1]
    (innermost stride-1 keeps the fast DVE path; plain stride-0
    broadcast is ~5x slower on HW).
  - Taps pre-added per di into g=10 group tensors on DVE (all packed
    adds); PE shift-accumulates groups into PSUM (bf16 identity
    matmuls, fully hidden per ablation). Pool gets one group (Pool is
    ~10x slower per op; keep its share tiny).
  - Column chunks of 64: psum [124,64,64] f32 = 8 banks (bufs=1); ACT
    psum->sbuf bf16 copy; output staged bf16, host upcasts.
Final 4 output rows (252 = 2*124 + 4) use the transposed scheme
(partition = output column, dj via 5 x copies, di on the free axis).
"""

import numpy as np

H = 256
W = 256
C = 64
K = 5
HO = H - K + 1  # 252
WO = W - K + 1  # 252
NCORES = 8
JW = 64  # column chunk width; psum tile [124, JW, C] f32 = 8 PSUM banks
BANK_J = 8  # 8 cols x 64 ch = 512 f32 = one PSUM bank
I0S = (0, 124)
J0S = tuple(range(0, WO, JW))  # (0, 64, 128, 192); last chunk jw=60
NCHUNK = len(I0S) * len(J0S)

# Tap grouping per di (dj indices per group). Each group: products
# pre-added on one engine, then one PE shift-accumulate stream.
GROUPS = {
    0: [[0, 1], [2], [3], [4]],
    1: [[0, 1], [2], [3], [4]],
    2: [[0, 1], [2], [3], [4]],
    3: [[0, 1], [2], [3], [4]],
    4: [[0, 1], [2], [3], [4]],
}
N_STREAMS = sum(len(g) for g in GROUPS.values())  # 20
# Whole groups on Pool (Pool op ~10x DVE op; give it a small share).
POOL_GROUPS = {(0, 2), (0, 3)}

_nc_cache = {}
ABLATE = "none"  # none|nomm|noeng|dmaonly (ablate.py)


def _build(reps=1):
    """reps>1 wraps the whole body in a HW loop (timing calibration only)."""
    global _nc_cache
    key = (reps, ABLATE)
    if key in _nc_cache:
        return _nc_cache[key]

    import contextlib

    import concourse.bacc as bacc
    import concourse.tile as tile
    from concourse import mybir
    from concourse.masks import make_identity

    f32 = mybir.dt.float32
    bf16 = mybir.dt.bfloat16
    mult = mybir.AluOpType.mult
    add = mybir.AluOpType.add

    nc = bacc.Bacc(None, target_bir_lowering=False)
    x = nc.dram_tensor("x", [H, W, C], bf16, kind="ExternalInput")
    fdc = nc.dram_tensor(
        "fdc", [NCHUNK, 128, K, JW, 2 * K], bf16, kind="ExternalInput"
    )
    fds = nc.dram_tensor(
        "fds", [2, 126, 4, 2 * K * K], bf16, kind="ExternalInput"
    )
    xst = nc.dram_tensor(
        "xst", [2, 126, K, 8, C], bf16, kind="ExternalInput"
    )
    out = nc.dram_tensor("out", [HO, WO, C], bf16, kind="ExternalOutput")
    outs = nc.dram_tensor("outs", [2, 126, 4, C], bf16, kind="ExternalOutput")

    with tile.TileContext(nc) as tc:
        with (
            tc.tile_pool(name="cst", bufs=1) as cst,
            tc.tile_pool(name="xp", bufs=3) as xp,
            tc.tile_pool(name="fp", bufs=3) as fp,
            tc.tile_pool(name="td", bufs=1) as td,
            tc.tile_pool(name="sp", bufs=3) as sp,
            tc.tile_pool(name="pp", bufs=1, space="PSUM") as pp,
        ):
            ident = cst.tile([128, 128], bf16, tag="ident")
            make_identity(nc, ident)

            with tc.For_i(0, reps, 1) if reps > 1 else contextlib.nullcontext():
                # --- main blocks: out rows [0,124) and [124,248) ---
                for bi, i0 in enumerate(I0S):
                    for ji, j0 in enumerate(J0S):
                        ci = bi * len(J0S) + ji
                        jw = min(JW, WO - j0)
                        xw = min(jw + K - 1, W - j0)
                        xt = xp.tile([128, JW + K - 1, C], bf16, tag="x")
                        nc.sync.dma_start(
                            out=xt[:, :xw, :],
                            in_=x[i0 : i0 + 128, j0 : j0 + xw, :],
                        )
                        # ftc[k, di, j, :] = dup'd taps di*K..di*K+4 of
                        # flow[i0+k-di, j0+j] (host-packed; edges zeroed)
                        ftc = fp.tile([128, K, JW, 2 * K], bf16, tag="f")
                        nc.sync.dma_start(
                            out=ftc, in_=fdc[ci, :, :, :, :]
                        )

                        ps = pp.tile([124, JW, C], f32, tag="ps")
                        if ABLATE in ("nomm", "noeng", "dmaonly"):
                            nc.vector.memset(ps[:1, :1, :], 0.0)
                        stream = 0
                        for di in range(K):
                            for gi, taps in enumerate(GROUPS[di]):
                                pool_op = (di, gi) in POOL_GROUPS
                                eng = nc.gpsimd if pool_op else nc.vector
                                tag = "gp" if pool_op else "gd"
                                g = td.tile([128, JW, C], bf16, tag=tag, bufs=4)

                                def fb(dj):
                                    return (
                                        ftc[:, di, :jw, 2 * dj : 2 * dj + 2]
                                        .unsqueeze(2)
                                        .to_broadcast([128, jw, C // 2, 2])
                                    )

                                if ABLATE not in ("noeng", "dmaonly"):
                                    eng.tensor_tensor(
                                        out=g[:, :jw, :],
                                        in0=xt[:, taps[0] : taps[0] + jw, :],
                                        in1=fb(taps[0]),
                                        op=mult,
                                    )
                                    for dj in taps[1:]:
                                        tb = td.tile(
                                            [128, JW, C], bf16,
                                            tag="tb", bufs=2,
                                        )
                                        eng.tensor_tensor(
                                            out=tb[:, :jw, :],
                                            in0=xt[:, dj : dj + jw, :],
                                            in1=fb(dj),
                                            op=mult,
                                        )
                                        g2 = td.tile(
                                            [128, JW, C], bf16,
                                            tag=tag, bufs=4,
                                        )
                                        eng.tensor_tensor(
                                            out=g2[:, :jw, :],
                                            in0=g[:, :jw, :],
                                            in1=tb[:, :jw, :],
                                            op=add,
                                        )
                                        g = g2
                                else:
                                    nc.gpsimd.memset(g[:1, :1, :], 0.0)
                                if ABLATE in ("none", "nodve"):
                                    for jj in range(0, jw, BANK_J):
                                        njw = min(BANK_J, jw - jj)
                                        nc.tensor.matmul(
                                            ps[:, jj : jj + njw, :],
                                            ident[:, di : di + 124],
                                            g[:, jj : jj + njw, :],
                                            start=(stream == 0),
                                            stop=(stream == N_STREAMS - 1),
                                        )
                                stream += 1
                        stage = sp.tile([124, JW, C], bf16, tag="stage")
                        nc.scalar.copy(out=stage[:, :jw, :], in_=ps[:, :jw, :])
                        nc.sync.dma_start(
                            out=out[i0 : i0 + 124, j0 : j0 + jw, :],
                            in_=stage[:, :jw, :],
                        )

                # --- strip: out rows [248,252), transposed (partition=j)
                # 2 blocks of 126 columns; host-packed inputs and output:
                # xst[si, p, dj, r, c] = x[HO-4+r, 126*si+p+dj, c]
                # fds[si, p, i, :] = dup'd flow[HO-4+i, 126*si+p, :]
                # outs[si, p, i, c] = out[HO-4+i, 126*si+p, c]
                for si in range(2):
                    P = 126
                    xs = fp.tile([P, K, 8, C], bf16, tag="sx")
                    nc.sync.dma_start(out=xs, in_=xst[si, :, :, :, :])
                    fs = fp.tile([P, 4, 2 * K * K], bf16, tag="sf")
                    nc.sync.dma_start(out=fs, in_=fds[si, :, :, :])
                    ps_s = pp.tile([P, 4, C], f32, tag="ps")
                    if ABLATE in ("nomm", "noeng", "dmaonly"):
                        nc.vector.memset(ps_s[:1, :1, :], 0.0)
                    for t in range(K * K):
                        di, dj = divmod(t, K)
                        if ABLATE in ("noeng", "dmaonly"):
                            continue
                        tmp = td.tile([P, 4, C], bf16, tag="st", bufs=4)
                        fbs = (
                            fs[:, :, 2 * t : 2 * t + 2]
                            .unsqueeze(2)
                            .to_broadcast([P, 4, C // 2, 2])
                        )
                        nc.vector.tensor_tensor(
                            out=tmp,
                            in0=xs[:, dj, di : di + 4, :],
                            in1=fbs,
                            op=mult,
                        )
                        if ABLATE == "none":
                            nc.tensor.matmul(
                                ps_s[:, :, :],
                                ident[:P, :P],
                                tmp[:, :, :],
                                start=(t == 0),
                                stop=(t == K * K - 1),
                            )
                    sstage = sp.tile([P, 4, C], bf16, tag="sstage")
                    nc.scalar.copy(out=sstage, in_=ps_s)
                    nc.sync.dma_start(out=outs[si, :, :, :], in_=sstage)

    nc.finalize()
    _nc_cache[key] = nc
    return nc


def _to_bf16(a):
    import ml_dtypes

    return np.ascontiguousarray(np.asarray(a).astype(ml_dtypes.bfloat16))


def _pack_flow(flow_core):
    """f32 [HO,WO,25] -> (fdc [NCHUNK,128,K,JW,2K], fds [4,128,4,50]) bf16."""
    import ml_dtypes

    fb = np.asarray(flow_core).astype(ml_dtypes.bfloat16)
    fdup = np.repeat(fb, 2, axis=-1)  # [HO, WO, 50]
    fdc = np.zeros((NCHUNK, 128, K, JW, 2 * K), dtype=ml_dtypes.bfloat16)
    for bi, i0 in enumerate(I0S):
        for ji, j0 in enumerate(J0S):
            ci = bi * len(J0S) + ji
            jw = min(JW, WO - j0)
            for di in range(K):
                lo = i0 - di
                ks = max(0, -lo)  # first valid k
                rows = fdup[
                    max(lo, 0) : lo + 128, j0 : j0 + jw,
                    2 * K * di : 2 * K * (di + 1),
                ]
                fdc[ci, ks : ks + rows.shape[0], di, :jw, :] = rows
    fds = np.zeros((2, 126, 4, 2 * K * K), dtype=ml_dtypes.bfloat16)
    for si in range(2):
        j0 = 126 * si
        # [4 rows, 126 cols, 50] -> [126, 4, 50]
        fds[si] = fdup[HO - 4 : HO, j0 : j0 + 126, :].transpose(1, 0, 2)
    return fdc, fds


def _pack_xst(x_core):
    """f32 [H,W,C] -> xst [2, 126, K, 8, C] bf16:
    xst[si, p, dj, r, c] = x[HO-4+r, 126*si+p+dj, c]."""
    import ml_dtypes

    xb = np.asarray(x_core).astype(ml_dtypes.bfloat16)
    xst = np.zeros((2, 126, K, 8, C), dtype=ml_dtypes.bfloat16)
    for si in range(2):
        j0 = 126 * si
        for dj in range(K):
            # [8 rows, 126 cols, C] -> [126, 8, C]
            xst[si, :, dj, :, :] = xb[
                HO - 4 : HO + 4, j0 + dj : j0 + dj + 126, :
            ].transpose(1, 0, 2)
    return xst


def _core_inputs(x_core, flow_core):
    """f32 [H,W,C] and [HO,WO,25] -> bf16 input map for one core."""
    fdc, fds = _pack_flow(flow_core)
    return {
        "x": _to_bf16(x_core),
        "fdc": np.ascontiguousarray(fdc),
        "fds": np.ascontiguousarray(fds),
        "xst": np.ascontiguousarray(_pack_xst(x_core)),
    }


def _postprocess_core(out_core, outs_core):
    o = np.asarray(out_core, dtype=np.float32)
    s = np.asarray(outs_core, dtype=np.float32)  # [2, 126, 4, C]
    for si in range(2):
        j0 = 126 * si
        o[HO - 4 : HO, j0 : j0 + 126, :] = s[si].transpose(1, 0, 2)
    return o


def _run(x, flow, trace=False):
    """x: [8,H,W,C] f32, flow: [8,HO,WO,25] f32 -> (out [8,HO,WO,C], res)"""
    from concourse.bass_utils import run_bass_kernel_spmd

    nc = _build()
    in_maps = [_core_inputs(x[b], flow[b]) for b in range(NCORES)]
    res = run_bass_kernel_spmd(
        nc, in_maps, core_ids=list(range(NCORES)), trace=trace
    )
    out = np.stack(
        [_postprocess_core(r["out"], r["outs"]) for r in res.results],
        axis=0,
    )
    return out, res


def kernel(x, flow, ksize=None, **_unused):
    x = np.asarray(x, dtype=np.float32)
    flow = np.asarray(flow, dtype=np.float32)
    out, _ = _run(x, flow, trace=False)
    return out



# revision 2
# speedup vs baseline: 4.2738x; 4.2738x over previous
"""Dynamic filter layer on 8 trn2 NeuronCores — v9 (PE patch-matmul).

out[b,i,j,c] = sum_{di,dj} x[b,i+di,j+dj,c] * flow[b,i,j,di*K+dj]

B=8, H=W=256, C=64, K=5, Ho=Wo=252. Sharding: data-parallel over batch,
one sample per core (SPMD, no collectives).

v9 design: map the whole dynamic filter onto TensorE as real matmuls.
Tile the output into 4x12-pixel patches. A patch reads an 8x16 input
window = 128 (row, col) sites -> the PE contraction dim. Per patch:

  stationary lhsT = x-window  [k=(a,jp)=128, m=c=64]      (bf16)
  moving rhs      = staircase [k=(a,jp), n=(r,jo)=48]     (bf16)
  psum out        = [m=c, n=(r,jo)]                        (f32)

  stair[(a,jp),(r,jo)] = flow[i0+r, j0+jo, (a-r)*K + (jp-jo)]
                         if 0<=a-r<K and 0<=jp-jo<K else 0

One N=48 matmul computes 48 pixels x 64 channels x all 25 taps (1600
useful MACs/cycle). 256 rows = 63 row-patches * 4 + 4 halo; 252 cols =
21 col-patches * 12: zero edge cases. Both operands are host-packed
(x windows 2.67x inflated, staircase 5.1x) -> ~46 MB/core DMA, the
roofline term. PE: 1323 LDW(64col)+MM(N=48) pairs/core.

Pipeline: super-rows of 3 row-patches per input DMA (1MB x / 0.77MB
stair); per row-patch a [64, 3banks] psum tile takes 21 MMs (7 patches
x 48 px x 64ch = 336 f32 per bank), evacuated f32->bf16 by DVE/ACT
(alternating) into a [128, ...] stage (row-patch supers pack pairwise
into the partition halves), DMA'd out per super-pair. Host unscrambles
the patch-major output layout.
"""

import numpy as np

H = 256
W = 256
C = 64
K = 5
HO = H - K + 1  # 252
WO = W - K + 1  # 252
NCORES = 8

R = 4  # output rows per patch
JT = 12  # output cols per patch
PA = R + K - 1  # 8 input rows per patch window
PJ = JT + K - 1  # 16 input cols per patch window
NPI = HO // R  # 63 row-patches
NPJ = WO // JT  # 21 col-patches
SUP = 3  # row-patches per input DMA super-row
NSUP = NPI // SUP  # 21
GPR = 3  # psum groups per row-patch
PPG = NPJ // GPR  # 7 patches per group (7*48=336 f32 <= 1 bank)
GF = PPG * R * JT  # 336 f32 per group
NOUT = (NSUP + 1) // 2  # 11 output super-pair slots

_nc_cache = {}
ABLATE = "none"  # none|nomm|noevac|dmaonly


def _build(reps=1):
    """reps>1 wraps the whole body in a HW loop (timing calibration only)."""
    global _nc_cache
    key = (reps, ABLATE)
    if key in _nc_cache:
        return _nc_cache[key]

    import contextlib

    import concourse.bacc as bacc
    import concourse.tile as tile
    from concourse import mybir

    f32 = mybir.dt.float32
    bf16 = mybir.dt.bfloat16

    nc = bacc.Bacc(None, target_bir_lowering=False)
    xpd = nc.dram_tensor(
        "xpd", [NSUP, 128, SUP, NPJ, C], bf16, kind="ExternalInput"
    )
    std = nc.dram_tensor(
        "std", [NSUP, 128, SUP, NPJ, R * JT], bf16, kind="ExternalInput"
    )
    outd = nc.dram_tensor(
        "outd", [NOUT, 128, SUP, GPR, GF], bf16, kind="ExternalOutput"
    )

    with tile.TileContext(nc) as tc:
        with (
            tc.tile_pool(name="xp", bufs=3) as xp,
            tc.tile_pool(name="fp", bufs=3) as fp,
            tc.tile_pool(name="op", bufs=2) as op,
            tc.tile_pool(name="pp", bufs=2, space="PSUM") as pp,
        ):
            with tc.For_i(0, reps, 1) if reps > 1 else contextlib.nullcontext():
                stage = None
                for s in range(NSUP):
                    xt = xp.tile([128, SUP, NPJ, C], bf16, tag="x")
                    nc.sync.dma_start(out=xt, in_=xpd[s, :, :, :, :])
                    st = fp.tile([128, SUP, NPJ, R * JT], bf16, tag="f")
                    nc.sync.dma_start(out=st, in_=std[s, :, :, :, :])
                    if s % 2 == 0:
                        stage = op.tile(
                            [128, SUP, GPR, GF], bf16, tag="stage"
                        )
                    pb = 64 * (s % 2)
                    for ro in range(SUP):
                        ps = pp.tile([C, GPR, 512], f32, tag="ps")
                        if ABLATE in ("noevac", "dmaonly"):
                            nc.vector.memset(ps[:, :, :1], 0.0)
                        if ABLATE != "dmaonly":
                            for pa in range(NPJ):
                                g, t = divmod(pa, PPG)
                                nc.tensor.matmul(
                                    ps[:, g, 48 * t : 48 * t + 48],
                                    xt[:, ro, pa, :],
                                    st[:, ro, pa, :],
                                    start=True,
                                    stop=True,
                                )
                        if ABLATE in ("nomm",) or ABLATE == "none":
                            eng_v = (s * SUP + ro) % 2 == 0
                            dst = stage[pb : pb + 64, ro, :, :]
                            src = ps[:, :, :GF]
                            if eng_v:
                                nc.vector.tensor_copy(dst, src)
                            else:
                                nc.scalar.copy(out=dst, in_=src)
                        elif ABLATE in ("noevac", "dmaonly"):
                            nc.vector.memset(
                                stage[pb : pb + 64, ro, :1, :1], 0.0
                            )
                    if s % 2 == 1:
                        nc.sync.dma_start(
                            out=outd[s // 2, :, :, :, :], in_=stage
                        )
                    elif s == NSUP - 1:
                        nc.sync.dma_start(
                            out=outd[s // 2, :64, :, :, :],
                            in_=stage[:64, :, :, :],
                        )

    nc.finalize()
    _nc_cache[key] = nc
    return nc


def _bf16():
    import ml_dtypes

    return ml_dtypes.bfloat16


def _pack_x(x_core):
    """f32 [H,W,C] -> xpd bf16 [NSUP, 128, SUP, NPJ, C]:
    xpd[s, a*PJ+jp, ro, pj, c] = x[R*(SUP*s+ro)+a, JT*pj+jp, c]."""
    bf16 = _bf16()
    xb = np.ascontiguousarray(np.asarray(x_core).astype(bf16))
    s0, s1, s2 = xb.strides
    win = np.lib.stride_tricks.as_strided(
        xb,
        shape=(NPI, PA, NPJ, PJ, C),
        strides=(R * s0, s0, JT * s1, s1, s2),
    )
    # [pi, a, pj, jp, c] -> [pi, (a,jp)=128, pj, c]
    xpr = win.transpose(0, 1, 3, 2, 4).reshape(NPI, 128, NPJ, C)
    xpd = (
        xpr.reshape(NSUP, SUP, 128, NPJ, C)
        .transpose(0, 2, 1, 3, 4)
    )
    return np.ascontiguousarray(xpd)


def _pack_flow(flow_core):
    """f32 [HO,WO,K*K] -> std bf16 [NSUP, 128, SUP, NPJ, R*JT]:
    std[s, (r+di)*PJ + jo+dj, ro, pj, r*JT+jo]
        = flow[R*(SUP*s+ro)+r, JT*pj+jo, di*K+dj]."""
    bf16 = _bf16()
    fb = np.ascontiguousarray(np.asarray(flow_core).astype(bf16))
    s0, s1, s2 = fb.strides
    fw = np.lib.stride_tricks.as_strided(
        fb,
        shape=(NPI, R, NPJ, JT, K * K),
        strides=(R * s0, s0, JT * s1, s1, s2),
    )
    # fw[pi, r, pj, jo, t]
    strp = np.zeros((NPI, 128, NPJ, R * JT), dtype=bf16)
    rr, jj = np.meshgrid(np.arange(R), np.arange(JT), indexing="ij")
    col = (rr * JT + jj).ravel()  # (48,)
    for t in range(K * K):
        di, dj = divmod(t, K)
        pidx = ((rr + di) * PJ + (jj + dj)).ravel()  # (48,)
        src = fw[:, :, :, :, t].transpose(0, 1, 3, 2).reshape(NPI, R * JT, NPJ)
        # advanced indexing on dims 1 and 3 -> result dims [48, NPI, NPJ]
        strp[:, pidx, :, col] = src.transpose(1, 0, 2)
    std = (
        strp.reshape(NSUP, SUP, 128, NPJ, R * JT)
        .transpose(0, 2, 1, 3, 4)
    )
    return np.ascontiguousarray(std)


def _core_inputs(x_core, flow_core):
    return {"xpd": _pack_x(x_core), "std": _pack_flow(flow_core)}


def _postprocess_core(outd_core):
    """bf16 [NOUT, 128, SUP, GPR, GF] -> f32 [HO, WO, C]."""
    o = np.asarray(outd_core, dtype=np.float32)
    # [q, (par, c), ro, g, (t, r, jo)]
    o = o.reshape(NOUT, 2, C, SUP, GPR, PPG, R, JT)
    # -> [q, par, ro, r, g, t, jo, c]
    o = o.transpose(0, 1, 3, 6, 4, 5, 7, 2)
    # s' = 2q+par in [0, 22); keep s' < NSUP
    o = o.reshape(NOUT * 2, SUP, R, GPR * PPG, JT, C)[:NSUP]
    # i = ((s*SUP)+ro)*R + r ; j = pa*JT + jo
    return np.ascontiguousarray(o.reshape(HO, WO, C))


def _run(x, flow, trace=False):
    """x: [8,H,W,C] f32, flow: [8,HO,WO,25] f32 -> (out [8,HO,WO,C], res)"""
    from concourse.bass_utils import run_bass_kernel_spmd

    nc = _build()
    in_maps = [_core_inputs(x[b], flow[b]) for b in range(NCORES)]
    res = run_bass_kernel_spmd(
        nc, in_maps, core_ids=list(range(NCORES)), trace=trace
    )
    out = np.stack(
        [_postprocess_core(r["outd"]) for r in res.results], axis=0
    )
    return out, res


def kernel(x, flow, ksize=None, **_unused):
    x = np.asarray(x, dtype=np.float32)
    flow = np.asarray(flow, dtype=np.float32)
    out, _ = _run(x, flow, trace=False)
    return out


# revision 3
# speedup vs baseline: 4.4615x; 1.0439x over previous
"""Dynamic filter layer on 8 trn2 NeuronCores — v9 (PE patch-matmul).

out[b,i,j,c] = sum_{di,dj} x[b,i+di,j+dj,c] * flow[b,i,j,di*K+dj]

B=8, H=W=256, C=64, K=5, Ho=Wo=252. Sharding: data-parallel over batch,
one sample per core (SPMD, no collectives).

v9 design: map the whole dynamic filter onto TensorE as real matmuls.
Tile the output into 4x12-pixel patches. A patch reads an 8x16 input
window = 128 (row, col) sites -> the PE contraction dim. Per patch:

  stationary lhsT = x-window  [k=(a,jp)=128, m=c=64]      (bf16)
  moving rhs      = staircase [k=(a,jp), n=(r,jo)=48]     (bf16)
  psum out        = [m=c, n=(r,jo)]                        (f32)

  stair[(a,jp),(r,jo)] = flow[i0+r, j0+jo, (a-r)*K + (jp-jo)]
                         if 0<=a-r<K and 0<=jp-jo<K else 0

One N=48 matmul computes 48 pixels x 64 channels x all 25 taps (1600
useful MACs/cycle). 256 rows = 63 row-patches * 4 + 4 halo; 252 cols =
21 col-patches * 12: zero edge cases. Both operands are host-packed
(x windows 2.67x inflated, staircase 5.1x) -> ~46 MB/core DMA, the
roofline term. PE: 1323 LDW(64col)+MM(N=48) pairs/core.

Pipeline: super-rows of 3 row-patches per input DMA (1MB x / 0.77MB
stair); per row-patch a [64, 3banks] psum tile takes 21 MMs (7 patches
x 48 px x 64ch = 336 f32 per bank), evacuated f32->bf16 by DVE/ACT
(alternating) into a [128, ...] stage (row-patch supers pack pairwise
into the partition halves), DMA'd out per super-pair. Host unscrambles
the patch-major output layout.
"""

import numpy as np

H = 256
W = 256
C = 64
K = 5
HO = H - K + 1  # 252
WO = W - K + 1  # 252
NCORES = 8

R = 4  # output rows per patch
JT = 12  # output cols per patch
PA = R + K - 1  # 8 input rows per patch window
PJ = JT + K - 1  # 16 input cols per patch window
NPI = HO // R  # 63 row-patches
NPJ = WO // JT  # 21 col-patches
SUP = 9  # row-patches per input DMA super-row
NSUP = NPI // SUP  # 21
GPR = 3  # psum groups per row-patch
PPG = NPJ // GPR  # 7 patches per group (7*48=336 f32 <= 1 bank)
GF = PPG * R * JT  # 336 f32 per group
NOUT = (NSUP + 1) // 2  # 11 output super-pair slots

_nc_cache = {}
ABLATE = "none"  # none|nomm|noevac|dmaonly


def _build(reps=1):
    """reps>1 wraps the whole body in a HW loop (timing calibration only)."""
    global _nc_cache
    key = (reps, ABLATE)
    if key in _nc_cache:
        return _nc_cache[key]

    import contextlib

    import concourse.bacc as bacc
    import concourse.tile as tile
    from concourse import mybir

    f32 = mybir.dt.float32
    bf16 = mybir.dt.bfloat16

    nc = bacc.Bacc(None, target_bir_lowering=False)
    xpd = nc.dram_tensor(
        "xpd", [NSUP, 128, SUP, NPJ, C], bf16, kind="ExternalInput"
    )
    std = nc.dram_tensor(
        "std", [NSUP, 128, SUP, NPJ, R * JT], bf16, kind="ExternalInput"
    )
    outd = nc.dram_tensor(
        "outd", [NOUT, 128, SUP, GPR, GF], bf16, kind="ExternalOutput"
    )

    with tile.TileContext(nc) as tc:
        with (
            tc.tile_pool(name="xp", bufs=3) as xp,
            tc.tile_pool(name="fp", bufs=3) as fp,
            tc.tile_pool(name="op", bufs=2) as op,
            tc.tile_pool(name="pp", bufs=2, space="PSUM") as pp,
        ):
            with tc.For_i(0, reps, 1) if reps > 1 else contextlib.nullcontext():
                stage = None
                for s in range(NSUP):
                    xt = xp.tile([128, SUP, NPJ, C], bf16, tag="x")
                    nc.sync.dma_start(out=xt, in_=xpd[s, :, :, :, :])
                    st = fp.tile([128, SUP, NPJ, R * JT], bf16, tag="f")
                    nc.sync.dma_start(out=st, in_=std[s, :, :, :, :])
                    if s % 2 == 0:
                        stage = op.tile(
                            [128, SUP, GPR, GF], bf16, tag="stage"
                        )
                    pb = 64 * (s % 2)
                    for ro in range(SUP):
                        ps = pp.tile([C, GPR, 512], f32, tag="ps")
                        if ABLATE in ("noevac", "dmaonly"):
                            nc.vector.memset(ps[:, :, :1], 0.0)
                        if ABLATE != "dmaonly":
                            for pa in range(NPJ):
                                g, t = divmod(pa, PPG)
                                nc.tensor.matmul(
                                    ps[:, g, 48 * t : 48 * t + 48],
                                    xt[:, ro, pa, :],
                                    st[:, ro, pa, :],
                                    start=True,
                                    stop=True,
                                )
                        if ABLATE in ("nomm",) or ABLATE == "none":
                            eng_v = (s * SUP + ro) % 2 == 0
                            dst = stage[pb : pb + 64, ro, :, :]
                            src = ps[:, :, :GF]
                            if eng_v:
                                nc.vector.tensor_copy(dst, src)
                            else:
                                nc.scalar.copy(out=dst, in_=src)
                        elif ABLATE in ("noevac", "dmaonly"):
                            nc.vector.memset(
                                stage[pb : pb + 64, ro, :1, :1], 0.0
                            )
                    if s % 2 == 1:
                        nc.sync.dma_start(
                            out=outd[s // 2, :, :, :, :], in_=stage
                        )
                    elif s == NSUP - 1:
                        nc.sync.dma_start(
                            out=outd[s // 2, :64, :, :, :],
                            in_=stage[:64, :, :, :],
                        )

    nc.finalize()
    _nc_cache[key] = nc
    return nc


def _bf16():
    import ml_dtypes

    return ml_dtypes.bfloat16


def _pack_x(x_core):
    """f32 [H,W,C] -> xpd bf16 [NSUP, 128, SUP, NPJ, C]:
    xpd[s, a*PJ+jp, ro, pj, c] = x[R*(SUP*s+ro)+a, JT*pj+jp, c]."""
    bf16 = _bf16()
    xb = np.ascontiguousarray(np.asarray(x_core).astype(bf16))
    s0, s1, s2 = xb.strides
    win = np.lib.stride_tricks.as_strided(
        xb,
        shape=(NPI, PA, NPJ, PJ, C),
        strides=(R * s0, s0, JT * s1, s1, s2),
    )
    # [pi, a, pj, jp, c] -> [pi, (a,jp)=128, pj, c]
    xpr = win.transpose(0, 1, 3, 2, 4).reshape(NPI, 128, NPJ, C)
    xpd = (
        xpr.reshape(NSUP, SUP, 128, NPJ, C)
        .transpose(0, 2, 1, 3, 4)
    )
    return np.ascontiguousarray(xpd)


def _pack_flow(flow_core):
    """f32 [HO,WO,K*K] -> std bf16 [NSUP, 128, SUP, NPJ, R*JT]:
    std[s, (r+di)*PJ + jo+dj, ro, pj, r*JT+jo]
        = flow[R*(SUP*s+ro)+r, JT*pj+jo, di*K+dj]."""
    bf16 = _bf16()
    fb = np.ascontiguousarray(np.asarray(flow_core).astype(bf16))
    s0, s1, s2 = fb.strides
    fw = np.lib.stride_tricks.as_strided(
        fb,
        shape=(NPI, R, NPJ, JT, K * K),
        strides=(R * s0, s0, JT * s1, s1, s2),
    )
    # fw[pi, r, pj, jo, t]
    strp = np.zeros((NPI, 128, NPJ, R * JT), dtype=bf16)
    rr, jj = np.meshgrid(np.arange(R), np.arange(JT), indexing="ij")
    col = (rr * JT + jj).ravel()  # (48,)
    for t in range(K * K):
        di, dj = divmod(t, K)
        pidx = ((rr + di) * PJ + (jj + dj)).ravel()  # (48,)
        src = fw[:, :, :, :, t].transpose(0, 1, 3, 2).reshape(NPI, R * JT, NPJ)
        # advanced indexing on dims 1 and 3 -> result dims [48, NPI, NPJ]
        strp[:, pidx, :, col] = src.transpose(1, 0, 2)
    std = (
        strp.reshape(NSUP, SUP, 128, NPJ, R * JT)
        .transpose(0, 2, 1, 3, 4)
    )
    return np.ascontiguousarray(std)


def _core_inputs(x_core, flow_core):
    return {"xpd": _pack_x(x_core), "std": _pack_flow(flow_core)}


def _postprocess_core(outd_core):
    """bf16 [NOUT, 128, SUP, GPR, GF] -> f32 [HO, WO, C]."""
    o = np.asarray(outd_core, dtype=np.float32)
    # [q, (par, c), ro, g, (t, r, jo)]
    o = o.reshape(NOUT, 2, C, SUP, GPR, PPG, R, JT)
    # -> [q, par, ro, r, g, t, jo, c]
    o = o.transpose(0, 1, 3, 6, 4, 5, 7, 2)
    # s' = 2q+par in [0, 22); keep s' < NSUP
    o = o.reshape(NOUT * 2, SUP, R, GPR * PPG, JT, C)[:NSUP]
    # i = ((s*SUP)+ro)*R + r ; j = pa*JT + jo
    return np.ascontiguousarray(o.reshape(HO, WO, C))


def _run(x, flow, trace=False):
    """x: [8,H,W,C] f32, flow: [8,HO,WO,25] f32 -> (out [8,HO,WO,C], res)"""
    from concourse.bass_utils import run_bass_kernel_spmd

    nc = _build()
    in_maps = [_core_inputs(x[b], flow[b]) for b in range(NCORES)]
    res = run_bass_kernel_spmd(
        nc, in_maps, core_ids=list(range(NCORES)), trace=trace
    )
    out = np.stack(
        [_postprocess_core(r["outd"]) for r in res.results], axis=0
    )
    return out, res


def kernel(x, flow, ksize=None, **_unused):
    x = np.asarray(x, dtype=np.float32)
    flow = np.asarray(flow, dtype=np.float32)
    out, _ = _run(x, flow, trace=False)
    return out
